# revision 1
# baseline (speedup 1.0000x reference)
"""Distributed Trainium2 Bass kernel for nn_Attention_25460566131147.

Multi-head attention (B=4, TQ=T=2048, E=2048, H=16, D=128) with gather-based
RoPE and key masking, sharded over 8 NeuronCores: data-parallel over batch
(4 groups) x tensor-parallel over heads (2-way: Wq/Wk/Wv column shards,
Wo row shards, AllReduce of the out-projection partials inside each pair).

Device algorithm (per core, all matmuls bf16 with f32 PSUM accumulation):
  - activations are kept feature-on-partitions (x^T layouts, prepared on host)
  - scores are computed transposed (S^T[k,q] = K_h^T-slice^T @ Q_h^T) so the
    exp'd tile P^T feeds the P@V matmul directly (no on-chip transposes)
  - exp via ScalarE activation with the key-mask folded into the per-partition
    bias and the 1/sqrt(D) scale folded into the activation scale; softmax max-
    subtraction is skipped (scores are O(6), fp32 exp is exact enough)
  - softmax denominator via a ones-column matmul accumulated alongside P@V;
    reciprocal on VectorE once; broadcast back via DMA (engines are 128-lane
    lockstep and cannot cross partitions; DMA can)
  - rotate-half for RoPE via two small SBUF->SBUF DMAs (partition rotation)
"""

import os
import sys

if "JAX_PLATFORMS" in os.environ and os.environ["JAX_PLATFORMS"] == "axon":
    os.environ["JAX_PLATFORMS"] = "axon,cpu"
sys.path.insert(0, "/opt/trn_rl_repo")

import numpy as np
import ml_dtypes

BF16NP = ml_dtypes.bfloat16

B, TQ, T, E, H, D = 4, 2048, 2048, 2048, 16, 128
BLOCK, THETA = 4096, 10000.0
N_CORES = 8
P = 128

FULL_CFG = dict(TQ=TQ, T=T, E=E, HL=8, D=D, NCORES=N_CORES)


def _cs(total, w):
    """Column splits: list of (start, width)."""
    return [(i, min(w, total - i)) for i in range(0, total, w)]


def build_nc(cfg=None):
    """Build and return the (uncompiled) Bacc graph for one SPMD core."""
    import concourse.mybir as mybir
    import concourse.tile as tile
    from concourse import bacc
    from contextlib import ExitStack

    c = dict(FULL_CFG)
    if cfg:
        c.update(cfg)
    cTQ, cT, cE, HL, cD, NCORES = (
        c["TQ"], c["T"], c["E"], c["HL"], c["D"], c["NCORES"],
    )
    assert cD == P
    F = HL * cD              # local feature width (heads shard)
    EC = cE // P             # contraction chunks for projections
    TC = cT // P             # key-position chunks
    NQ = min(512, cTQ)       # q-tile width (PSUM bank limit)
    BF = mybir.dt.bfloat16
    F32 = mybir.dt.float32
    SCALE = 1.0 / float(np.sqrt(cD))
    groups = [[2 * i, 2 * i + 1] for i in range(NCORES // 2)]

    nc = bacc.Bacc("TRN2", target_bir_lowering=False, debug=False,
                   num_devices=NCORES)

    xt_d = nc.declare_dram_parameter("xt", [cE, cTQ], BF, isOutput=False)
    xat_d = nc.declare_dram_parameter("xat", [cE, cT], BF, isOutput=False)
    wq_d = nc.declare_dram_parameter("wq", [cE, F], BF, isOutput=False)
    wk_d = nc.declare_dram_parameter("wk", [cE, F], BF, isOutput=False)
    wv_d = nc.declare_dram_parameter("wv", [cE, F], BF, isOutput=False)
    wo_d = nc.declare_dram_parameter("wo", [F, cE], BF, isOutput=False)
    cosq_d = nc.declare_dram_parameter("cosq", [P, cTQ], BF, isOutput=False)
    sinq_d = nc.declare_dram_parameter("sinq", [P, cTQ], BF, isOutput=False)
    cosk_d = nc.declare_dram_parameter("cosk", [P, cT], BF, isOutput=False)
    sink_d = nc.declare_dram_parameter("sink", [P, cT], BF, isOutput=False)
    mb_d = nc.declare_dram_parameter("mbias", [P, TC], F32, isOutput=False)
    NT = cE // P                      # out-projection row tiles
    NCH = 8 if NT % 8 == 0 else (2 if NT % 2 == 0 else 1)
    CR = (NT // NCH) * P              # chunk rows (collective granule)
    out_d = nc.declare_dram_parameter("out", [cE // 2, cTQ], BF, isOutput=True)

    obounce = [nc.dram_tensor(f"obounce{ch}", [CR, cTQ], BF)
               for ch in range(NCH)]
    orsc = [nc.dram_tensor(f"orsc{ch}", [CR // 2, cTQ], BF)
            for ch in range(NCH)]

    with tile.TileContext(nc) as tc, ExitStack() as ex:
        # right side: persistent accumulating tiles; left side: phase-scoped
        consts = ex.enter_context(tc.tile_pool(name="consts", bufs=1, side="right"))
        ones_bf = consts.tile([P, 1], BF, tag="ones_bf", name="ones_bf")
        nc.vector.memset(ones_bf[:], 1.0)
        mb_sb = consts.tile([P, TC], F32, tag="mbias", name="mbias")
        nc.sync.dma_start(mb_sb[:], mb_d[:])
        # packed denominators: head m lives at partition base (m%4)*32
        # (engine ops need 32-aligned start partitions), column (m//4)*128
        den_sb = consts.tile([P, 2 * P], F32, tag="den", name="den")
        ones_fr = consts.tile([1, P], F32, tag="ones_fr", name="ones_fr")
        nc.vector.memset(ones_fr[:], 1.0)

        SEG = min(512, cT)   # projection/rope column-segment width

        def proj_rope(m, w_sb, src_tiles, src_c0, out_c0, width, cos_sb,
                      sin_sb, out_t, tg, rawp, tmpp, psproj):
            """Project head m (cols [src_c0, src_c0+width) of src) and apply
            RoPE, writing cols [out_c0, out_c0+width) of out_t."""
            raw = rawp.tile([P, width], BF, tag=f"raw{tg}", name=f"raw{tg}")
            swp = rawp.tile([P, width], BF, tag=f"swp{tg}", name=f"swp{tg}")
            ps = psproj.tile([P, SEG], F32, tag="projps", name="projps")
            for e in range(EC):
                for ns, nw in _cs(width, 512):
                    nc.tensor.matmul(
                        ps[:, ns:ns + nw],
                        w_sb[e][:, m * P:(m + 1) * P],
                        src_tiles[e][:, src_c0 + ns:src_c0 + ns + nw],
                        start=(e == 0), stop=(e == EC - 1),
                    )
            nc.scalar.copy(raw[:], ps[:, 0:width])
            # partition rotate-half via SBUF->SBUF DMA (cross-partition)
            half = P // 2
            nc.sync.dma_start(swp[0:half, :], raw[half:P, :])
            nc.sync.dma_start(swp[half:P, :], raw[0:half, :])
            t1 = tmpp.tile([P, width], BF, tag="rope_t1", name="rope_t1")
            t2 = tmpp.tile([P, width], BF, tag="rope_t2", name="rope_t2")
            nc.vector.tensor_mul(t1[:], raw[:], cos_sb[:, out_c0:out_c0 + width])
            nc.vector.tensor_mul(t2[:], swp[:], sin_sb[:, out_c0:out_c0 + width])
            nc.vector.tensor_add(out_t[:, out_c0:out_c0 + width], t1[:], t2[:])

        vp = ex.enter_context(tc.tile_pool(name="v", bufs=1, side="right"))
        ktp = ex.enter_context(tc.tile_pool(name="kt", bufs=1, side="right"))
        qtp = ex.enter_context(tc.tile_pool(name="qt", bufs=1, side="right"))

        es_proj = ExitStack()   # projection psum: [V .. Q]
        psproj = es_proj.enter_context(
            tc.tile_pool(name="psproj", bufs=2, space="PSUM"))
        # prefetch pools opened early so their DMA loads are not WAR-blocked
        # behind the previous phase's tiles
        es_xt = ExitStack()     # x^T quarters: [V .. Q]
        xtp = es_xt.enter_context(tc.tile_pool(name="xt", bufs=1))
        es_xak = ExitStack()    # xall^T quarters for K: [V .. K]
        xakp = es_xak.enter_context(tc.tile_pool(name="xak", bufs=1))

        # ============ phase V: V = xall @ Wv, [t-part, n-free] ===========
        # xall^T is streamed in 1024-column halves (and re-streamed for K)
        # to bound SBUF.
        assert F <= 1024
        v_sb = [vp.tile([P, F], BF, tag=f"v{t}", name=f"v{t}")
                for t in range(TC)]
        with tc.tile_pool(name="wv", bufs=1) as wvp, \
                tc.tile_pool(name="xav", bufs=1) as xavp:
            wv_sb = []
            for e in range(EC):
                t_ = wvp.tile([P, F], BF, tag=f"wv{e}", name=f"wv{e}")
                nc.sync.dma_start(t_[:], wv_d[e * P:(e + 1) * P, :])
                wv_sb.append(t_)
            for h0, hw in _cs(cT, SEG):
                xa_sb = []
                for e in range(EC):
                    t_ = xavp.tile([P, SEG], BF, tag=f"xav{e}", name=f"xav{e}")
                    nc.sync.dma_start(
                        t_[:, 0:hw], xat_d[e * P:(e + 1) * P, h0:h0 + hw])
                    xa_sb.append(t_)
                for tl in range(hw // P):
                    t = (h0 // P) + tl
                    ps = psproj.tile([P, F], F32, tag="projpsv", name="projpsv")
                    for e in range(EC):
                        for ns, nw in _cs(F, 512):
                            nc.tensor.matmul(
                                ps[:, ns:ns + nw],
                                xa_sb[e][:, tl * P:(tl + 1) * P],
                                wv_sb[e][:, ns:ns + nw],
                                start=(e == 0), stop=(e == EC - 1),
                            )
                    nc.scalar.copy(v_sb[t][:], ps[:, 0:F])

        # ============ phase K: K-proj + RoPE =============================
        kt_sb = [ktp.tile([P, cT], BF, tag=f"kt{m}", name=f"kt{m}")
                 for m in range(HL)]
        with tc.tile_pool(name="tabk", bufs=1) as tabk, \
                tc.tile_pool(name="wk", bufs=1) as wkp, \
                tc.tile_pool(name="rawk", bufs=1) as rawkp, \
                tc.tile_pool(name="tmpk", bufs=2) as tmpkp:
            cosk_sb = tabk.tile([P, cT], BF, tag="cosk", name="cosk")
            sink_sb = tabk.tile([P, cT], BF, tag="sink", name="sink")
            nc.sync.dma_start(cosk_sb[:], cosk_d[:])
            nc.sync.dma_start(sink_sb[:], sink_d[:])
            wk_sb = []
            for e in range(EC):
                t_ = wkp.tile([P, F], BF, tag=f"wk{e}", name=f"wk{e}")
                nc.sync.dma_start(t_[:], wk_d[e * P:(e + 1) * P, :])
                wk_sb.append(t_)
            for h0, hw in _cs(cT, SEG):
                xa_sb = []
                for e in range(EC):
                    t_ = xakp.tile([P, SEG], BF, tag=f"xak{e}", name=f"xak{e}")
                    nc.sync.dma_start(
                        t_[:, 0:hw], xat_d[e * P:(e + 1) * P, h0:h0 + hw])
                    xa_sb.append(t_)
                for m in range(HL):
                    proj_rope(m, wk_sb, xa_sb, 0, h0, hw, cosk_sb,
                              sink_sb, kt_sb[m], "k", rawkp, tmpkp, psproj)
        es_xak.close()

        # ============ phase Q: Q-proj + RoPE (x^T in halves) =============
        qt_sb = []
        for m in range(HL):
            qt_sb.append(qtp.tile([P, cTQ], BF, tag=f"qt{m}", name=f"qt{m}"))
        with tc.tile_pool(name="wq", bufs=1) as wqp, \
                tc.tile_pool(name="tabq", bufs=1) as tabq, \
                tc.tile_pool(name="rawq", bufs=1) as rawqp, \
                tc.tile_pool(name="tmpq", bufs=2) as tmpqp:
            cosq_sb = tabq.tile([P, cTQ], BF, tag="cosq", name="cosq")
            sinq_sb = tabq.tile([P, cTQ], BF, tag="sinq", name="sinq")
            nc.sync.dma_start(cosq_sb[:], cosq_d[:])
            nc.sync.dma_start(sinq_sb[:], sinq_d[:])
            wq_sb = []
            for e in range(EC):
                t_ = wqp.tile([P, F], BF, tag=f"wq{e}", name=f"wq{e}")
                nc.sync.dma_start(t_[:], wq_d[e * P:(e + 1) * P, :])
                wq_sb.append(t_)
            TQH = min(512, cTQ)
            for th, (h0, hw) in enumerate(_cs(cTQ, TQH)):
                xt_sb = []
                for e in range(EC):
                    t_ = xtp.tile([P, TQH], BF, tag=f"xt{e}", name=f"xt{e}")
                    nc.sync.dma_start(
                        t_[:], xt_d[e * P:(e + 1) * P, h0:h0 + hw])
                    xt_sb.append(t_)
                for m in range(HL):
                    proj_rope(m, wq_sb, xt_sb, 0, h0, hw, cosq_sb, sinq_sb,
                              qt_sb[m], "q", rawqp, tmpqp, psproj)
        es_xt.close()
        es_proj.close()

        # ================= phase C: attention ============================
        es_yt = ExitStack()     # yt tiles: [C .. end of D] (normalized in place)
        ytp = es_yt.enter_context(tc.tile_pool(name="yt", bufs=1))
        yt_sb = []
        for m in range(HL):
            yt_sb.append(ytp.tile([P, cTQ], BF, tag=f"yt{m}", name=f"yt{m}"))

        FR = mybir.dt.float32r
        RPM = cTQ // P                # packed den rows per head
        with tc.tile_pool(name="pt", bufs=TC + 2) as ptp, \
                tc.tile_pool(name="pt2", bufs=TC // 2 + 2) as pt2p, \
                tc.tile_pool(name="dst", bufs=2) as dstp, \
                tc.tile_pool(name="dner", bufs=2) as dnerp, \
                tc.tile_pool(name="pss", bufs=3, space="PSUM") as pss, \
                tc.tile_pool(name="psy", bufs=2, space="PSUM") as psy, \
                tc.tile_pool(name="psd", bufs=2, space="PSUM") as psd, \
                tc.tile_pool(name="psb", bufs=1, space="PSUM") as psb:
            for m in range(HL):
                for qs, qw in _cs(cTQ, NQ):
                    yps = psy.tile([P, NQ], F32, tag="yps", name="yps")
                    dps = psd.tile([1, NQ], F32, tag="dps", name="dps")
                    pts = []
                    for kc in range(TC):
                        sps = pss.tile([P, NQ], F32, tag="sps", name="sps")
                        nc.tensor.matmul(
                            sps[:, 0:qw],
                            kt_sb[m][:, kc * P:(kc + 1) * P],
                            qt_sb[m][:, qs:qs + qw],
                            start=True, stop=True,
                        )
                        pt = ptp.tile([P, NQ], BF, tag="pt", name="pt")
                        pts.append(pt)
                        nc.scalar.activation(
                            pt[:, 0:qw], sps[:, 0:qw],
                            mybir.ActivationFunctionType.Exp,
                            bias=mb_sb[:, kc:kc + 1], scale=SCALE,
                        )
                        nc.tensor.matmul(
                            yps[:, 0:qw],
                            v_sb[kc][:, m * P:(m + 1) * P],
                            pt[:, 0:qw],
                            start=(kc == 0), stop=(kc == TC - 1),
                        )
                    # denominator: pair-sum the exp'd chunks on the idle
                    # GpSimd/Vector engines, then TC/2 ones-matmuls
                    assert TC % 2 == 0
                    pt2s = []
                    for i in range(TC // 2):
                        pt2 = pt2p.tile([P, NQ], BF, tag="pt2", name="pt2")
                        pt2s.append(pt2)
                        eng = nc.gpsimd if (i % 2 == 0) else nc.vector
                        eng.tensor_add(pt2[:, 0:qw], pts[2 * i][:, 0:qw],
                                       pts[2 * i + 1][:, 0:qw])
                    for i in range(TC // 2):
                        nc.tensor.matmul(
                            dps[0:1, 0:qw],
                            ones_bf[:, 0:1],
                            pt2s[i][:, 0:qw],
                            start=(i == 0), stop=(i == TC // 2 - 1),
                        )
                    nc.vector.tensor_copy(yt_sb[m][:, qs:qs + qw], yps[:, 0:qw])
                    dst = dstp.tile([1, NQ], F32, tag="dst", name="dst")
                    nc.vector.tensor_copy(dst[0:1, 0:qw], dps[0:1, 0:qw])
                    # scatter the denominator row into the packed layout
                    # (DMA can cross partitions)
                    bp = (m % 4) * 32 + qs // P
                    c0 = (m // 4) * P
                    nc.sync.dma_start(
                        den_sb[bp:bp + qw // P, c0:c0 + P], dst[0:1, 0:qw])
                # ---- head m normalization (overlaps head m+1 attention) ----
                bp = (m % 4) * 32
                c0 = (m // 4) * P
                nc.vector.reciprocal(den_sb[bp:bp + RPM, c0:c0 + P],
                                     den_sb[bp:bp + RPM, c0:c0 + P])
                dner = dnerp.tile([1, cTQ], F32, tag="dner", name="dner")
                nc.sync.dma_start(dner[0:1, :],
                                  den_sb[bp:bp + RPM, c0:c0 + P])
                for qs, qw in _cs(cTQ, NQ):
                    dbc = psb.tile([P, NQ], F32, tag="dbc", name="dbc")
                    nc.tensor.matmul(
                        dbc[:, 0:qw],
                        ones_fr[0:1, :].bitcast(FR),
                        dner[0:1, qs:qs + qw].bitcast(FR),
                        start=True, stop=True,
                    )
                    nc.vector.tensor_mul(
                        yt_sb[m][:, qs:qs + qw],
                        yt_sb[m][:, qs:qs + qw],
                        dbc[:, 0:qw],
                    )

        es_wo = ExitStack()     # wo tiles: [D]
        wop = es_wo.enter_context(tc.tile_pool(name="wo", bufs=1))
        wo_sb = []
        for f in range(HL):
            t_ = wop.tile([P, cE], BF, tag=f"wo{f}", name=f"wo{f}")
            nc.sync.dma_start(t_[:], wo_d[f * P:(f + 1) * P, :])
            wo_sb.append(t_)

        # ================= phase D: out-projection =======================
        # emitted in NCH chunks of CR rows of E; each chunk's partials are
        # ReduceScattered within the pair while the next chunk computes
        with tc.tile_pool(name="oev", bufs=4) as oevp, \
                tc.tile_pool(name="pso", bufs=2, space="PSUM") as pso:
            for ch in range(NCH):
                for nl in range(CR // P):
                    n = ch * (CR // P) + nl
                    for ms, mw in _cs(cTQ, 512):
                        ops = pso.tile([P, 512], F32, tag="ops", name="ops")
                        for f in range(HL):
                            nc.tensor.matmul(
                                ops[:, 0:mw],
                                wo_sb[f][:, n * P:(n + 1) * P],
                                yt_sb[f][:, ms:ms + mw],
                                start=(f == 0), stop=(f == HL - 1),
                            )
                        oev = oevp.tile([P, 512], BF, tag="oev", name="oev")
                        nc.scalar.copy(oev[:, 0:mw], ops[:, 0:mw])
                        nc.sync.dma_start(
                            obounce[ch][nl * P:(nl + 1) * P, ms:ms + mw],
                            oev[:, 0:mw])
                nc.gpsimd.collective_compute(
                    "ReduceScatter",
                    mybir.AluOpType.add,
                    replica_groups=groups,
                    ins=[obounce[ch][:]],
                    outs=[orsc[ch][:]],
                )
                nc.sync.dma_start(
                    out_d[ch * (CR // 2):(ch + 1) * (CR // 2), :],
                    orsc[ch][:])
        es_wo.close()
        es_yt.close()

    return nc




# ---------------------------------------------------------------------------
# host side
# ---------------------------------------------------------------------------

def _rope_tables():
    inv_freq = 1.0 / (THETA ** (np.arange(0, D, 2, dtype=np.float32) / D))
    t = np.arange(BLOCK, dtype=np.float32)
    freqs = np.einsum("i,j->ij", t, inv_freq).astype(np.float32)
    emb = np.concatenate([freqs, freqs], axis=-1)
    return np.cos(emb).astype(np.float32), np.sin(emb).astype(np.float32)


_NC_CACHE = {}


def _get_compiled():
    if "nc" not in _NC_CACHE:
        nc = build_nc()
        nc.compile()
        _NC_CACHE["nc"] = nc
    return _NC_CACHE["nc"]


def _bf(a):
    return np.ascontiguousarray(a).astype(BF16NP)


def prepare_in_maps(x, xall, posx, posxall, mask, Wq, Wk, Wv, Wo):
    x = np.asarray(x, dtype=np.float32)
    xall = np.asarray(xall, dtype=np.float32)
    posx = np.asarray(posx)
    posxall = np.asarray(posxall)
    mask = np.asarray(mask)
    Wq = np.asarray(Wq, dtype=np.float32)
    Wk = np.asarray(Wk, dtype=np.float32)
    Wv = np.asarray(Wv, dtype=np.float32)
    Wo = np.asarray(Wo, dtype=np.float32)

    cos_t, sin_t = _rope_tables()
    sign = np.ones((1, D), np.float32)
    sign[0, : D // 2] = -1.0

    F = (H * D) // 2  # 1024: per-core head-shard width

    in_maps = []
    for c in range(N_CORES):
        b, hg = c // 2, c % 2
        sl = slice(hg * F, (hg + 1) * F)
        cosq = _bf(cos_t[posx[b]].T)                    # [128, TQ]
        sinq = _bf((sin_t[posx[b]] * sign).T)
        cosk = _bf(cos_t[posxall[b]].T)
        sink = _bf((sin_t[posxall[b]] * sign).T)
        mb = np.where(mask[b], np.float32(-60.0), np.float32(0.0))
        mb = np.ascontiguousarray(mb.reshape(T // P, P).T)  # [128, TC]
        in_maps.append({
            "xt": _bf(x[b].T),
            "xat": _bf(xall[b].T),
            "wq": _bf(Wq[:, sl]),
            "wk": _bf(Wk[:, sl]),
            "wv": _bf(Wv[:, sl]),
            "wo": _bf(Wo[sl, :]),
            "cosq": cosq, "sinq": sinq, "cosk": cosk, "sink": sink,
            "mbias": mb.astype(np.float32),
        })
    return in_maps


def assemble_out(results):
    # ReduceScatter within each pair: chunk ch covers E rows
    # [ch*CR, (ch+1)*CR); rank hg holds the half [ch*CR + hg*CR/2, ...).
    NT = E // P
    NCH = 8 if NT % 8 == 0 else (2 if NT % 2 == 0 else 1)
    CR = (NT // NCH) * P
    out = np.empty((B, TQ, E), np.float32)
    outT = np.empty((E, TQ), np.float32)
    for b in range(B):
        for hg in range(2):
            half = results[2 * b + hg]["out"].astype(np.float32)
            for ch in range(NCH):
                outT[ch * CR + hg * (CR // 2):ch * CR + (hg + 1) * (CR // 2)] = \
                    half[ch * (CR // 2):(ch + 1) * (CR // 2)]
        out[b] = outT.T
    return out


def kernel(x, xall, posx, posxall, mask, Wq, Wk, Wv, Wo):
    from concourse.bass_utils import run_bass_kernel_spmd

    in_maps = prepare_in_maps(x, xall, posx, posxall, mask, Wq, Wk, Wv, Wo)
    nc = _get_compiled()
    res = run_bass_kernel_spmd(nc, in_maps, list(range(N_CORES)), trace=False)
    return assemble_out(res.results)



# revision 6
# speedup vs baseline: 1.0484x; 1.0484x over previous
"""Distributed Trainium2 Bass kernel for nn_Attention_25460566131147.

Multi-head attention (B=4, TQ=T=2048, E=2048, H=16, D=128) with gather-based
RoPE and key masking, sharded over 8 NeuronCores: data-parallel over batch
(4 groups) x tensor-parallel over heads (2-way: Wq/Wk/Wv column shards).

Out-projection strategy (v2): instead of each core computing a full-E partial
out-projection and ReduceScattering at the end (leaves ~150us of collectives
on the critical path), each core AllGathers the normalized per-head attention
outputs yt within its pair DURING the attention phase (two AllGathers, fired
after head 3 and head 7), then computes the out-projection for its half of
the E output features with the FULL H*D contraction. The AllGather layout
induces a fixed head-block permutation of the features, which is folded into
a host-side permutation of Wo's rows (rank-independent). The out-projection
reads all 16 head blocks from the gathered DRAM buffers; the contraction is
ordered first-half-first so the second AllGather is hidden behind ~27us of
matmuls on already-arrived data.

Device algorithm (per core, all matmuls bf16 with f32 PSUM accumulation):
  - activations are kept feature-on-partitions (x^T layouts, prepared on host)
  - scores are computed transposed (S^T[k,q] = K_h^T-slice^T @ Q_h^T) so the
    exp'd tile P^T feeds the P@V matmul directly (no on-chip transposes)
  - exp via ScalarE activation with the key-mask folded into the per-partition
    bias and the 1/sqrt(D) scale folded into the activation scale; softmax max-
    subtraction is skipped (scores are O(6), fp32 exp is exact enough)
  - softmax denominator via a binary add-tree of the exp'd tiles on the
    Vector/GpSimd engines plus a single ones-column matmul (instead of TC/2
    ones-matmuls: saves ~47us of TensorE time); reciprocal on VectorE once;
    broadcast back via a float32r ones matmul
  - rotate-half for RoPE via two small SBUF->SBUF DMAs (partition rotation)
  - next-phase weights/tables are prefetched during the previous phase so the
    TensorE never waits on DMA at phase boundaries (also keeps the PE HAM
    clock-gate warm: idle gaps >3.4us re-throttle the array to 1.2GHz)

SBUF pools follow the tile allocator's strict LIFO discipline per side; a
pool's full footprint spans open..close, so prefetch pools are opened in
reverse order of their close times.
"""

import os
import sys

if "JAX_PLATFORMS" in os.environ and os.environ["JAX_PLATFORMS"] == "axon":
    os.environ["JAX_PLATFORMS"] = "axon,cpu"
sys.path.insert(0, "/opt/trn_rl_repo")

import numpy as np
import ml_dtypes

BF16NP = ml_dtypes.bfloat16

B, TQ, T, E, H, D = 4, 2048, 2048, 2048, 16, 128
BLOCK, THETA = 4096, 10000.0
N_CORES = 8
P = 128

FULL_CFG = dict(TQ=TQ, T=T, E=E, HL=8, D=D, NCORES=N_CORES)


def _cs(total, w):
    """Column splits: list of (start, width)."""
    return [(i, min(w, total - i)) for i in range(0, total, w)]


def build_nc(cfg=None):
    """Build and return the (uncompiled) Bacc graph for one SPMD core."""
    import concourse.mybir as mybir
    import concourse.tile as tile
    from concourse import bacc
    from contextlib import ExitStack

    c = dict(FULL_CFG)
    if cfg:
        c.update(cfg)
    cTQ, cT, cE, HL, cD, NCORES = (
        c["TQ"], c["T"], c["E"], c["HL"], c["D"], c["NCORES"],
    )
    assert cD == P
    F = HL * cD              # local feature width (heads shard)
    EC = cE // P             # contraction chunks for projections
    TC = cT // P             # key-position chunks
    NQ = min(512, cTQ)       # q-tile width (PSUM bank limit)
    EH = cE // 2             # out-feature half owned by this core
    BF = mybir.dt.bfloat16
    F32 = mybir.dt.float32
    SCALE = 1.0 / float(np.sqrt(cD))
    groups = [[2 * i, 2 * i + 1] for i in range(NCORES // 2)]

    nc = bacc.Bacc("TRN2", target_bir_lowering=False, debug=False,
                   num_devices=NCORES)

    xt_d = nc.declare_dram_parameter("xt", [cE, cTQ], BF, isOutput=False)
    xat_d = nc.declare_dram_parameter("xat", [cE, cT], BF, isOutput=False)
    wq_d = nc.declare_dram_parameter("wq", [cE, F], BF, isOutput=False)
    wk_d = nc.declare_dram_parameter("wk", [cE, F], BF, isOutput=False)
    wv_d = nc.declare_dram_parameter("wv", [cE, F], BF, isOutput=False)
    # host-permuted Wo rows (AllGather block order), this core's E-col half
    wo_d = nc.declare_dram_parameter("wo", [2 * F, EH], BF, isOutput=False)
    cosq_d = nc.declare_dram_parameter("cosq", [P, cTQ], BF, isOutput=False)
    sinq_d = nc.declare_dram_parameter("sinq", [P, cTQ], BF, isOutput=False)
    cosk_d = nc.declare_dram_parameter("cosk", [P, cT], BF, isOutput=False)
    sink_d = nc.declare_dram_parameter("sink", [P, cT], BF, isOutput=False)
    mb_d = nc.declare_dram_parameter("mbias", [P, TC], F32, isOutput=False)
    out_d = nc.declare_dram_parameter("out", [EH, cTQ], BF, isOutput=True)

    # yt exchange buffers: own halves, gathered pair halves
    HH = HL // 2             # heads per AllGather (4)
    agin = [nc.dram_tensor(f"agin{h}", [HH * P, cTQ], BF) for h in range(2)]
    agout = [nc.dram_tensor(f"agout{h}", [2 * HH * P, cTQ], BF)
             for h in range(2)]

    with tile.TileContext(nc) as tc, ExitStack() as ex:
        # right side: persistent accumulating tiles; left side: phase-scoped
        consts = ex.enter_context(tc.tile_pool(name="consts", bufs=1, side="right"))
        ones_bf = consts.tile([P, 1], BF, tag="ones_bf", name="ones_bf")
        nc.vector.memset(ones_bf[:], 1.0)
        mb_sb = consts.tile([P, TC], F32, tag="mbias", name="mbias")
        nc.sync.dma_start(mb_sb[:], mb_d[:])
        # packed denominators: head m lives at partition base (m%4)*32
        # (engine ops need 32-aligned start partitions), column (m//4)*128
        den_sb = consts.tile([P, 2 * P], F32, tag="den", name="den")
        ones_fr = consts.tile([1, P], F32, tag="ones_fr", name="ones_fr")
        nc.vector.memset(ones_fr[:], 1.0)

        vp = ex.enter_context(tc.tile_pool(name="v", bufs=1, side="right"))
        es_kqv = ExitStack()  # kt/qt pools: closed before phase D (SBUF reuse)

        # left-side pool stack, opened in reverse close order (LIFO):
        es_proj = ExitStack()   # projection psum: [V .. Q]
        psproj = es_proj.enter_context(
            tc.tile_pool(name="psproj", bufs=2, space="PSUM"))
        es_tabq = ExitStack()   # Q-phase weights+tables: [pre-V .. Q]
        wqp = es_tabq.enter_context(tc.tile_pool(name="wq", bufs=1))
        tabq = es_tabq.enter_context(tc.tile_pool(name="tabq", bufs=1))
        es_xt = ExitStack()     # x^T quarters: [pre-V .. Q]
        xtp = es_xt.enter_context(tc.tile_pool(name="xt", bufs=1))
        es_tabk = ExitStack()   # K-phase weights+tables: [pre-V .. K]
        wkp = es_tabk.enter_context(tc.tile_pool(name="wk", bufs=1))
        tabk = es_tabk.enter_context(tc.tile_pool(name="tabk", bufs=1))
        es_xak = ExitStack()    # xall^T quarters for K: [pre-V .. K]
        xakp = es_xak.enter_context(tc.tile_pool(name="xak", bufs=1))

        SEG = min(512, cT)   # projection/rope column-segment width

        def proj_rope(m, w_sb, src_tiles, src_c0, out_c0, width, cos_sb,
                      sin_sb, out_t, tg, rawp, tmpp):
            """Project head m (cols [src_c0, src_c0+width) of src) and apply
            RoPE, writing cols [out_c0, out_c0+width) of out_t."""
            raw = rawp.tile([P, width], BF, tag=f"raw{tg}", name=f"raw{tg}")
            swp = rawp.tile([P, width], BF, tag=f"swp{tg}", name=f"swp{tg}")
            ps = psproj.tile([P, SEG], F32, tag="projps", name="projps")
            for e in range(EC):
                for ns, nw in _cs(width, 512):
                    nc.tensor.matmul(
                        ps[:, ns:ns + nw],
                        w_sb[e][:, m * P:(m + 1) * P],
                        src_tiles[e][:, src_c0 + ns:src_c0 + ns + nw],
                        start=(e == 0), stop=(e == EC - 1),
                    )
            nc.scalar.copy(raw[:], ps[:, 0:width])
            # partition rotate-half via SBUF->SBUF DMA (cross-partition)
            half = P // 2
            nc.sync.dma_start(swp[0:half, :], raw[half:P, :])
            nc.sync.dma_start(swp[half:P, :], raw[0:half, :])
            t1 = tmpp.tile([P, width], BF, tag="rope_t1", name="rope_t1")
            t2 = tmpp.tile([P, width], BF, tag="rope_t2", name="rope_t2")
            nc.vector.tensor_mul(t1[:], raw[:], cos_sb[:, out_c0:out_c0 + width])
            nc.vector.tensor_mul(t2[:], swp[:], sin_sb[:, out_c0:out_c0 + width])
            nc.vector.tensor_add(out_t[:, out_c0:out_c0 + width], t1[:], t2[:])

        # ============ phase V: V = xall @ Wv, [t-part, n-free] ===========
        # xall^T is streamed in 512-column chunks (and re-streamed for K)
        # to bound SBUF.
        assert F <= 1024
        v_sb = [vp.tile([P, F], BF, tag=f"v{t}", name=f"v{t}")
                for t in range(TC)]
        with tc.tile_pool(name="wv", bufs=1) as wvp, \
                tc.tile_pool(name="xav", bufs=1) as xavp:
            wv_sb = []
            for e in range(EC):
                t_ = wvp.tile([P, F], BF, tag=f"wv{e}", name=f"wv{e}")
                wv_sb.append(t_)
            # first compute tile needs all xav e-chunks of seg 0 plus wv[0];
            # emit wv[0], then the seg-0 xav DMAs, then the rest of wv
            nc.sync.dma_start(wv_sb[0][:], wv_d[0:P, :])
            seg0_xa = []
            for e in range(EC):
                t_ = xavp.tile([P, SEG], BF, tag=f"xav{e}", name=f"xav{e}")
                nc.sync.dma_start(t_[:], xat_d[e * P:(e + 1) * P, 0:SEG])
                seg0_xa.append(t_)
            for e in range(1, EC):
                nc.sync.dma_start(wv_sb[e][:], wv_d[e * P:(e + 1) * P, :])
            # prefetch K-phase tables+weights (used next phase)
            cosk_sb = tabk.tile([P, cT], BF, tag="cosk", name="cosk")
            sink_sb = tabk.tile([P, cT], BF, tag="sink", name="sink")
            nc.sync.dma_start(cosk_sb[:], cosk_d[:])
            nc.sync.dma_start(sink_sb[:], sink_d[:])
            wk_sb = []
            for e in range(EC):
                t_ = wkp.tile([P, F], BF, tag=f"wk{e}", name=f"wk{e}")
                nc.sync.dma_start(t_[:], wk_d[e * P:(e + 1) * P, :])
                wk_sb.append(t_)
            for h0, hw in _cs(cT, SEG):
                if h0 == 0:
                    xa_sb = seg0_xa
                else:
                    xa_sb = []
                    for e in range(EC):
                        t_ = xavp.tile([P, SEG], BF, tag=f"xav{e}", name=f"xav{e}")
                        nc.sync.dma_start(
                            t_[:, 0:hw], xat_d[e * P:(e + 1) * P, h0:h0 + hw])
                        xa_sb.append(t_)
                for tl in range(hw // P):
                    t = (h0 // P) + tl
                    ps = psproj.tile([P, F], F32, tag="projpsv", name="projpsv")
                    for e in range(EC):
                        for ns, nw in _cs(F, 512):
                            nc.tensor.matmul(
                                ps[:, ns:ns + nw],
                                xa_sb[e][:, tl * P:(tl + 1) * P],
                                wv_sb[e][:, ns:ns + nw],
                                start=(e == 0), stop=(e == EC - 1),
                            )
                    nc.scalar.copy(v_sb[t][:], ps[:, 0:F])

        # ============ phase K: K-proj + RoPE =============================
        ktp = es_kqv.enter_context(tc.tile_pool(name="kt", bufs=1, side="right"))
        kt_sb = [ktp.tile([P, cT], BF, tag=f"kt{m}", name=f"kt{m}")
                 for m in range(HL)]
        with tc.tile_pool(name="rawk", bufs=1) as rawkp, \
                tc.tile_pool(name="tmpk", bufs=2) as tmpkp:
            first = True
            for h0, hw in _cs(cT, SEG):
                xa_sb = []
                for e in range(EC):
                    t_ = xakp.tile([P, SEG], BF, tag=f"xak{e}", name=f"xak{e}")
                    nc.sync.dma_start(
                        t_[:, 0:hw], xat_d[e * P:(e + 1) * P, h0:h0 + hw])
                    xa_sb.append(t_)
                if first:
                    # prefetch Q-phase tables+weights behind seg-0 loads
                    first = False
                    cosq_sb = tabq.tile([P, cTQ], BF, tag="cosq", name="cosq")
                    sinq_sb = tabq.tile([P, cTQ], BF, tag="sinq", name="sinq")
                    nc.sync.dma_start(cosq_sb[:], cosq_d[:])
                    nc.sync.dma_start(sinq_sb[:], sinq_d[:])
                    wq_sb = []
                    for e in range(EC):
                        t_ = wqp.tile([P, F], BF, tag=f"wq{e}", name=f"wq{e}")
                        nc.sync.dma_start(t_[:], wq_d[e * P:(e + 1) * P, :])
                        wq_sb.append(t_)
                for m in range(HL):
                    proj_rope(m, wk_sb, xa_sb, 0, h0, hw, cosk_sb,
                              sink_sb, kt_sb[m], "k", rawkp, tmpkp)
        es_xak.close()
        es_tabk.close()

        # ============ phase Q: Q-proj + RoPE (x^T in quarters) ===========
        qtp = es_kqv.enter_context(tc.tile_pool(name="qt", bufs=1, side="right"))
        qt_sb = []
        for m in range(HL):
            qt_sb.append(qtp.tile([P, cTQ], BF, tag=f"qt{m}", name=f"qt{m}"))
        with tc.tile_pool(name="rawq", bufs=1) as rawqp, \
                tc.tile_pool(name="tmpq", bufs=2) as tmpqp:
            TQH = min(512, cTQ)
            for th, (h0, hw) in enumerate(_cs(cTQ, TQH)):
                xt_sb = []
                for e in range(EC):
                    t_ = xtp.tile([P, TQH], BF, tag=f"xt{e}", name=f"xt{e}")
                    nc.sync.dma_start(
                        t_[:], xt_d[e * P:(e + 1) * P, h0:h0 + hw])
                    xt_sb.append(t_)
                for m in range(HL):
                    proj_rope(m, wq_sb, xt_sb, 0, h0, hw, cosq_sb, sinq_sb,
                              qt_sb[m], "q", rawqp, tmpqp)
        es_xt.close()
        es_tabq.close()
        es_proj.close()

        # ================= phase C: attention ============================
        FR = mybir.dt.float32r
        RPM = cTQ // P                # packed den rows per head
        es_wo = ExitStack()     # out-proj weights, loaded during attention
        wop = es_wo.enter_context(tc.tile_pool(name="wo", bufs=1))
        es_ya = ExitStack()     # first 4 gathered yt tiles (loaded in C)
        ya1p = es_ya.enter_context(tc.tile_pool(name="ya1", bufs=1))
        es_att = ExitStack()
        ptp = es_att.enter_context(tc.tile_pool(name="pt", bufs=5))
        pt2p = es_att.enter_context(tc.tile_pool(name="pt2", bufs=TC // 2 + 1))
        pt4p = es_att.enter_context(tc.tile_pool(name="pt4", bufs=2))
        pt8p = es_att.enter_context(tc.tile_pool(name="pt8", bufs=2))
        ytp = es_att.enter_context(tc.tile_pool(name="yt", bufs=2))
        dstp = es_att.enter_context(tc.tile_pool(name="dst", bufs=2))
        dnerp = es_att.enter_context(tc.tile_pool(name="dner", bufs=1))
        pss = es_att.enter_context(tc.tile_pool(name="pss", bufs=3, space="PSUM"))
        psy = es_att.enter_context(tc.tile_pool(name="psy", bufs=2, space="PSUM"))
        psd = es_att.enter_context(tc.tile_pool(name="psd", bufs=2, space="PSUM"))
        psb = es_att.enter_context(tc.tile_pool(name="psb", bufs=1, space="PSUM"))

        wo_sb = []
        for f in range(2 * F // P):
            t_ = wop.tile([P, EH], BF, tag=f"wo{f}", name=f"wo{f}")
            nc.sync.dma_start(t_[:], wo_d[f * P:(f + 1) * P, :])
            wo_sb.append(t_)

        ya_sb = []
        for m in range(HL):
            yt_t = ytp.tile([P, cTQ], BF, tag="yt", name=f"yt{m}")
            for qs, qw in _cs(cTQ, NQ):
                yps = psy.tile([P, NQ], F32, tag="yps", name="yps")
                dps = psd.tile([1, NQ], F32, tag="dps", name="dps")
                pt2s = []
                pts = []
                for kc in range(TC):
                    sps = pss.tile([P, NQ], F32, tag="sps", name="sps")
                    nc.tensor.matmul(
                        sps[:, 0:qw],
                        kt_sb[m][:, kc * P:(kc + 1) * P],
                        qt_sb[m][:, qs:qs + qw],
                        start=True, stop=True,
                    )
                    pt = ptp.tile([P, NQ], BF, tag="pt", name="pt")
                    pts.append(pt)
                    nc.scalar.activation(
                        pt[:, 0:qw], sps[:, 0:qw],
                        mybir.ActivationFunctionType.Exp,
                        bias=mb_sb[:, kc:kc + 1], scale=SCALE,
                    )
                    nc.tensor.matmul(
                        yps[:, 0:qw],
                        v_sb[kc][:, m * P:(m + 1) * P],
                        pt[:, 0:qw],
                        start=(kc == 0), stop=(kc == TC - 1),
                    )
                    if kc % 2 == 1:
                        # denominator add-tree level 1 (interleaved so pt
                        # tiles free early)
                        i = kc // 2
                        pt2 = pt2p.tile([P, NQ], BF, tag="pt2", name="pt2")
                        pt2s.append(pt2)
                        eng = nc.gpsimd if (i % 2 == 0) else nc.vector
                        eng.tensor_add(pt2[:, 0:qw], pts[kc - 1][:, 0:qw],
                                       pts[kc][:, 0:qw])
                # add-tree levels 2..4 on Vector/GpSimd, then one ones-matmul
                lvl = pt2s
                pools = {4: pt4p, 2: pt8p, 1: pt8p}
                j = 0
                while len(lvl) > 1:
                    nxt = []
                    pool = pools[len(lvl) // 2]
                    for i in range(len(lvl) // 2):
                        t_ = pool.tile([P, NQ], BF, tag=f"ptl{len(lvl)}_{i % 2}",
                                       name="ptl")
                        eng = nc.gpsimd if (j % 2 == 0) else nc.vector
                        eng.tensor_add(t_[:, 0:qw], lvl[2 * i][:, 0:qw],
                                       lvl[2 * i + 1][:, 0:qw])
                        nxt.append(t_)
                        j += 1
                    lvl = nxt
                nc.tensor.matmul(
                    dps[0:1, 0:qw],
                    ones_bf[:, 0:1],
                    lvl[0][:, 0:qw],
                    start=True, stop=True,
                )
                nc.vector.tensor_copy(yt_t[:, qs:qs + qw], yps[:, 0:qw])
                dst = dstp.tile([1, NQ], F32, tag="dst", name="dst")
                nc.vector.tensor_copy(dst[0:1, 0:qw], dps[0:1, 0:qw])
                # scatter the denominator row into the packed layout
                # (DMA can cross partitions)
                bp = (m % 4) * 32 + qs // P
                c0 = (m // 4) * P
                nc.sync.dma_start(
                    den_sb[bp:bp + qw // P, c0:c0 + P], dst[0:1, 0:qw])
            # ---- head m normalization (overlaps head m+1 attention) ----
            bp = (m % 4) * 32
            c0 = (m // 4) * P
            nc.vector.reciprocal(den_sb[bp:bp + RPM, c0:c0 + P],
                                 den_sb[bp:bp + RPM, c0:c0 + P])
            dner = dnerp.tile([1, cTQ], F32, tag="dner", name="dner")
            nc.sync.dma_start(dner[0:1, :],
                              den_sb[bp:bp + RPM, c0:c0 + P])
            for qs, qw in _cs(cTQ, NQ):
                dbc = psb.tile([P, NQ], F32, tag="dbc", name="dbc")
                nc.tensor.matmul(
                    dbc[:, 0:qw],
                    ones_fr[0:1, :].bitcast(FR),
                    dner[0:1, qs:qs + qw].bitcast(FR),
                    start=True, stop=True,
                )
                nc.vector.tensor_mul(
                    yt_t[:, qs:qs + qw],
                    yt_t[:, qs:qs + qw],
                    dbc[:, 0:qw],
                )
            # ship normalized head to the pair-exchange buffer
            half, ml = divmod(m, HH)
            nc.sync.dma_start(agin[half][ml * P:(ml + 1) * P, :], yt_t[:])
            if m == HH - 1:
                nc.gpsimd.collective_compute(
                    "AllGather",
                    mybir.AluOpType.bypass,
                    replica_groups=groups,
                    ins=[agin[0][:]],
                    outs=[agout[0][:]],
                )
            if m == HH:
                # first 4 gathered tiles: DMA in during heads 5-7
                for f in range(HH):
                    t_ = ya1p.tile([P, cTQ], BF, tag=f"ya{f}", name=f"ya{f}")
                    nc.sync.dma_start(t_[:], agout[0][f * P:(f + 1) * P, :])
                    ya_sb.append(t_)
        nc.gpsimd.collective_compute(
            "AllGather",
            mybir.AluOpType.bypass,
            replica_groups=groups,
            ins=[agin[1][:]],
            outs=[agout[1][:]],
        )
        es_att.close()
        es_kqv.close()

        # ================= phase D: out-projection =======================
        # out^T[EH, q] = Wo'^T @ [ya1; ya2]; f 0..7 (arrived via AG1) are
        # accumulated across all 8 E-row tiles first, hiding AG2 + its loads
        NT = EH // P
        with tc.tile_pool(name="ya2", bufs=1) as ya2p, \
                tc.tile_pool(name="oev", bufs=4) as oevp, \
                tc.tile_pool(name="pso", bufs=8, space="PSUM") as pso:
            for f in range(HH, 2 * HH):
                t_ = ya2p.tile([P, cTQ], BF, tag=f"yb{f}", name=f"yb{f}")
                nc.sync.dma_start(t_[:], agout[0][f * P:(f + 1) * P, :])
                ya_sb.append(t_)
            for f in range(2 * HH):
                t_ = ya2p.tile([P, cTQ], BF, tag=f"yc{f}", name=f"yc{f}")
                nc.sync.dma_start(t_[:], agout[1][f * P:(f + 1) * P, :])
                ya_sb.append(t_)
            for ms, mw in _cs(cTQ, 512):
                opss = []
                for n in range(NT):
                    ops = pso.tile([P, 512], F32, tag="ops", name="ops")
                    opss.append(ops)
                    for f in range(HL):
                        nc.tensor.matmul(
                            ops[:, 0:mw],
                            wo_sb[f][:, n * P:(n + 1) * P],
                            ya_sb[f][:, ms:ms + mw],
                            start=(f == 0), stop=False,
                        )
                for n in range(NT):
                    ops = opss[n]
                    for f in range(HL, 2 * HL):
                        nc.tensor.matmul(
                            ops[:, 0:mw],
                            wo_sb[f][:, n * P:(n + 1) * P],
                            ya_sb[f][:, ms:ms + mw],
                            start=False, stop=(f == 2 * HL - 1),
                        )
                    oev = oevp.tile([P, 512], BF, tag="oev", name="oev")
                    nc.scalar.copy(oev[:, 0:mw], ops[:, 0:mw])
                    nc.sync.dma_start(
                        out_d[n * P:(n + 1) * P, ms:ms + mw],
                        oev[:, 0:mw])
        es_ya.close()
        es_wo.close()

    return nc


# ---------------------------------------------------------------------------
# host side
# ---------------------------------------------------------------------------

def _rope_tables():
    inv_freq = 1.0 / (THETA ** (np.arange(0, D, 2, dtype=np.float32) / D))
    t = np.arange(BLOCK, dtype=np.float32)
    freqs = np.einsum("i,j->ij", t, inv_freq).astype(np.float32)
    emb = np.concatenate([freqs, freqs], axis=-1)
    return np.cos(emb).astype(np.float32), np.sin(emb).astype(np.float32)


_NC_CACHE = {}


def _get_compiled():
    if "nc" not in _NC_CACHE:
        nc = build_nc()
        nc.compile()
        _NC_CACHE["nc"] = nc
    return _NC_CACHE["nc"]


def _bf(a):
    return np.ascontiguousarray(a).astype(BF16NP)


def prepare_in_maps(x, xall, posx, posxall, mask, Wq, Wk, Wv, Wo):
    x = np.asarray(x, dtype=np.float32)
    xall = np.asarray(xall, dtype=np.float32)
    posx = np.asarray(posx)
    posxall = np.asarray(posxall)
    mask = np.asarray(mask)
    Wq = np.asarray(Wq, dtype=np.float32)
    Wk = np.asarray(Wk, dtype=np.float32)
    Wv = np.asarray(Wv, dtype=np.float32)
    Wo = np.asarray(Wo, dtype=np.float32)

    cos_t, sin_t = _rope_tables()
    sign = np.ones((1, D), np.float32)
    sign[0, : D // 2] = -1.0

    F = (H * D) // 2  # 1024: per-core head-shard width
    FH = F // 2       # 512: AllGather half (4 heads)
    # AllGather block order: [A h0-3, B h0-3, A h4-7, B h4-7] where A/B are
    # the pair's rank-0/rank-1 feature halves of Wo's rows
    Wo_perm = np.concatenate(
        [Wo[0:FH], Wo[F:F + FH], Wo[FH:F], Wo[F + FH:2 * F]], axis=0)

    in_maps = []
    for c in range(N_CORES):
        b, hg = c // 2, c % 2
        sl = slice(hg * F, (hg + 1) * F)
        cosq = _bf(cos_t[posx[b]].T)                    # [128, TQ]
        sinq = _bf((sin_t[posx[b]] * sign).T)
        cosk = _bf(cos_t[posxall[b]].T)
        sink = _bf((sin_t[posxall[b]] * sign).T)
        mb = np.where(mask[b], np.float32(-60.0), np.float32(0.0))
        mb = np.ascontiguousarray(mb.reshape(T // P, P).T)  # [128, TC]
        in_maps.append({
            "xt": _bf(x[b].T),
            "xat": _bf(xall[b].T),
            "wq": _bf(Wq[:, sl]),
            "wk": _bf(Wk[:, sl]),
            "wv": _bf(Wv[:, sl]),
            "wo": _bf(Wo_perm[:, hg * (E // 2):(hg + 1) * (E // 2)]),
            "cosq": cosq, "sinq": sinq, "cosk": cosk, "sink": sink,
            "mbias": mb.astype(np.float32),
        })
    return in_maps


def assemble_out(results):
    # core (b, hg) computed out^T for E columns [hg*E/2, (hg+1)*E/2)
    EH = E // 2
    out = np.empty((B, TQ, E), np.float32)
    for b in range(B):
        for hg in range(2):
            half = results[2 * b + hg]["out"].astype(np.float32)
            out[b][:, hg * EH:(hg + 1) * EH] = half.T
    return out


def kernel(x, xall, posx, posxall, mask, Wq, Wk, Wv, Wo):
    from concourse.bass_utils import run_bass_kernel_spmd

    in_maps = prepare_in_maps(x, xall, posx, posxall, mask, Wq, Wk, Wv, Wo)
    nc = _get_compiled()
    res = run_bass_kernel_spmd(nc, in_maps, list(range(N_CORES)), trace=False)
    return assemble_out(res.results)


# revision 14
# speedup vs baseline: 1.2239x; 1.1675x over previous
"""Distributed Trainium2 Bass kernel for nn_Attention_25460566131147.

Multi-head attention (B=4, TQ=T=2048, E=2048, H=16, D=128) with gather-based
RoPE and key masking, sharded over 8 NeuronCores: data-parallel over batch
(4 groups) x tensor-parallel over heads (2-way: Wq/Wk/Wv column shards).

Out-projection strategy (v2): instead of each core computing a full-E partial
out-projection and ReduceScattering at the end (leaves ~150us of collectives
on the critical path), each core AllGathers the normalized per-head attention
outputs yt within its pair DURING the attention phase (two AllGathers, fired
after head 3 and head 7), then computes the out-projection for its half of
the E output features with the FULL H*D contraction. The AllGather layout
induces a fixed head-block permutation of the features, which is folded into
a host-side permutation of Wo's rows (rank-independent). The out-projection
reads all 16 head blocks from the gathered DRAM buffers; the contraction is
ordered first-half-first so the second AllGather is hidden behind ~27us of
matmuls on already-arrived data.

Device algorithm (per core, all matmuls bf16 with f32 PSUM accumulation):
  - activations are kept feature-on-partitions (x^T layouts, prepared on host)
  - scores are computed transposed (S^T[k,q] = K_h^T-slice^T @ Q_h^T) so the
    exp'd tile P^T feeds the P@V matmul directly (no on-chip transposes)
  - exp via ScalarE activation with the key-mask folded into the per-partition
    bias and the 1/sqrt(D) scale folded into the activation scale; softmax max-
    subtraction is skipped (scores are O(6), fp32 exp is exact enough)
  - softmax denominator via a binary add-tree of the exp'd tiles on the
    Vector/GpSimd engines plus a single ones-column matmul (instead of TC/2
    ones-matmuls: saves ~47us of TensorE time); reciprocal on VectorE once;
    broadcast back via a float32r ones matmul
  - rotate-half for RoPE via two small SBUF->SBUF DMAs (partition rotation)
  - next-phase weights/tables are prefetched during the previous phase so the
    TensorE never waits on DMA at phase boundaries (also keeps the PE HAM
    clock-gate warm: idle gaps >3.4us re-throttle the array to 1.2GHz)

SBUF pools follow the tile allocator's strict LIFO discipline per side; a
pool's full footprint spans open..close, so prefetch pools are opened in
reverse order of their close times.
"""

import os
import sys

if "JAX_PLATFORMS" in os.environ and os.environ["JAX_PLATFORMS"] == "axon":
    os.environ["JAX_PLATFORMS"] = "axon,cpu"
sys.path.insert(0, "/opt/trn_rl_repo")

import numpy as np
import ml_dtypes

BF16NP = ml_dtypes.bfloat16

B, TQ, T, E, H, D = 4, 2048, 2048, 2048, 16, 128
BLOCK, THETA = 4096, 10000.0
N_CORES = 8
P = 128

FULL_CFG = dict(TQ=TQ, T=T, E=E, HL=8, D=D, NCORES=N_CORES)


def _cs(total, w):
    """Column splits: list of (start, width)."""
    return [(i, min(w, total - i)) for i in range(0, total, w)]


def build_nc(cfg=None):
    """Build and return the (uncompiled) Bacc graph for one SPMD core."""
    import concourse.mybir as mybir
    import concourse.tile as tile
    from concourse import bacc
    from contextlib import ExitStack

    c = dict(FULL_CFG)
    if cfg:
        c.update(cfg)
    cTQ, cT, cE, HL, cD, NCORES = (
        c["TQ"], c["T"], c["E"], c["HL"], c["D"], c["NCORES"],
    )
    assert cD == P
    F = HL * cD              # local feature width (heads shard)
    EC = cE // P             # contraction chunks for projections
    TC = cT // P             # key-position chunks
    NQ = min(512, cTQ)       # q-tile width (PSUM bank limit)
    EH = cE // 2             # out-feature half owned by this core
    BF = mybir.dt.bfloat16
    F32 = mybir.dt.float32
    SCALE = 1.0 / float(np.sqrt(cD))
    groups = [[2 * i, 2 * i + 1] for i in range(NCORES // 2)]

    nc = bacc.Bacc("TRN2", target_bir_lowering=False, debug=False,
                   num_devices=NCORES)

    xt_d = nc.declare_dram_parameter("xt", [cE, cTQ], BF, isOutput=False)
    xat_d = nc.declare_dram_parameter("xat", [cE, cT], BF, isOutput=False)
    wq_d = nc.declare_dram_parameter("wq", [cE, F], BF, isOutput=False)
    wk_d = nc.declare_dram_parameter("wk", [cE, F], BF, isOutput=False)
    wv_d = nc.declare_dram_parameter("wv", [cE, F], BF, isOutput=False)
    # host-permuted Wo rows (AllGather block order), this core's E-col half
    wo_d = nc.declare_dram_parameter("wo", [2 * F, EH], BF, isOutput=False)
    cosq_d = nc.declare_dram_parameter("cosq", [P, cTQ], BF, isOutput=False)
    sinq_d = nc.declare_dram_parameter("sinq", [P, cTQ], BF, isOutput=False)
    cosk_d = nc.declare_dram_parameter("cosk", [P, cT], BF, isOutput=False)
    sink_d = nc.declare_dram_parameter("sink", [P, cT], BF, isOutput=False)
    mb_d = nc.declare_dram_parameter("mbias", [P, TC], F32, isOutput=False)
    out_d = nc.declare_dram_parameter("out", [EH, cTQ], BF, isOutput=True)

    # yt exchange buffers: own 2-head blocks, gathered pair blocks
    HH = 2                   # heads per AllGather
    NAG = HL // HH           # number of AllGathers (4)
    agin = [nc.dram_tensor(f"agin{h}", [HH * P, cTQ], BF) for h in range(NAG)]
    agout = [nc.dram_tensor(f"agout{h}", [2 * HH * P, cTQ], BF)
             for h in range(NAG)]

    with tile.TileContext(nc) as tc, ExitStack() as ex:
        # right side: persistent accumulating tiles; left side: phase-scoped
        consts = ex.enter_context(tc.tile_pool(name="consts", bufs=1, side="right"))
        ones_bf = consts.tile([P, 1], BF, tag="ones_bf", name="ones_bf")
        nc.vector.memset(ones_bf[:], 1.0)
        mb_sb = consts.tile([P, TC], F32, tag="mbias", name="mbias")
        nc.sync.dma_start(mb_sb[:], mb_d[:])
        # packed denominators: head m lives at partition base (m%4)*32
        # (engine ops need 32-aligned start partitions), column (m//4)*128
        den_sb = consts.tile([P, 2 * P], F32, tag="den", name="den")
        ones_fr = consts.tile([1, P], F32, tag="ones_fr", name="ones_fr")
        nc.vector.memset(ones_fr[:], 1.0)

        vp = ex.enter_context(tc.tile_pool(name="v", bufs=1, side="right"))
        es_kqv = ExitStack()  # kt/qt pools: closed before phase D (SBUF reuse)

        # left-side pool stack, opened in reverse close order (LIFO):
        es_proj = ExitStack()   # projection psum: [V .. Q]
        warmp = es_proj.enter_context(
            tc.tile_pool(name="warm", bufs=1, space="PSUM"))
        psproj = es_proj.enter_context(
            tc.tile_pool(name="psproj", bufs=2, space="PSUM"))
        es_tabq = ExitStack()   # Q-phase weights+tables: [pre-V .. Q]
        wqp = es_tabq.enter_context(tc.tile_pool(name="wq", bufs=1))
        tabq = es_tabq.enter_context(tc.tile_pool(name="tabq", bufs=1))
        es_xt = ExitStack()     # x^T quarters: [pre-V .. Q]
        xtp = es_xt.enter_context(tc.tile_pool(name="xt", bufs=1))
        es_tabk = ExitStack()   # K-phase weights+tables: [pre-V .. K]
        wkp = es_tabk.enter_context(tc.tile_pool(name="wk", bufs=1))
        tabk = es_tabk.enter_context(tc.tile_pool(name="tabk", bufs=1))
        es_xak = ExitStack()    # xall^T quarters for K: [pre-V .. K]
        xakp = es_xak.enter_context(tc.tile_pool(name="xak", bufs=1))

        SEG = min(512, cT)   # projection/rope column-segment width

        def proj_rope(m, w_sb, src_tiles, src_c0, out_c0, width, cos_sb,
                      sin_sb, out_t, tg, rawp, tmpp):
            """Project head m (cols [src_c0, src_c0+width) of src) and apply
            RoPE, writing cols [out_c0, out_c0+width) of out_t."""
            raw = rawp.tile([P, width], BF, tag=f"raw{tg}", name=f"raw{tg}")
            swp = rawp.tile([P, width], BF, tag=f"swp{tg}", name=f"swp{tg}")
            ps = psproj.tile([P, SEG], F32, tag="projps", name="projps")
            for e in range(EC):
                for ns, nw in _cs(width, 512):
                    nc.tensor.matmul(
                        ps[:, ns:ns + nw],
                        w_sb[e][:, m * P:(m + 1) * P],
                        src_tiles[e][:, src_c0 + ns:src_c0 + ns + nw],
                        start=(e == 0), stop=(e == EC - 1),
                    )
            nc.scalar.copy(raw[:], ps[:, 0:width])
            # partition rotate-half via SBUF->SBUF DMA (cross-partition)
            half = P // 2
            nc.sync.dma_start(swp[0:half, :], raw[half:P, :])
            nc.sync.dma_start(swp[half:P, :], raw[0:half, :])
            t1 = tmpp.tile([P, width], BF, tag="rope_t1", name="rope_t1")
            t2 = tmpp.tile([P, width], BF, tag="rope_t2", name="rope_t2")
            nc.vector.tensor_mul(t1[:], raw[:], cos_sb[:, out_c0:out_c0 + width])
            nc.vector.tensor_mul(t2[:], swp[:], sin_sb[:, out_c0:out_c0 + width])
            nc.vector.tensor_add(out_t[:, out_c0:out_c0 + width], t1[:], t2[:])

        # ============ phase V: V = xall @ Wv, [t-part, n-free] ===========
        # xall^T is streamed in 512-column chunks (and re-streamed for K)
        # to bound SBUF.
        assert F <= 1024
        v_sb = [vp.tile([P, F], BF, tag=f"v{t}", name=f"v{t}")
                for t in range(TC)]
        with tc.tile_pool(name="wv", bufs=1) as wvp, \
                tc.tile_pool(name="xav", bufs=1) as xavp:
            wv_sb = []
            for e in range(EC):
                t_ = wvp.tile([P, F], BF, tag=f"wv{e}", name=f"wv{e}")
                wv_sb.append(t_)
            # first compute tile needs all xav e-chunks of seg 0 plus wv[0];
            # emit those DMAs first, then the rest of wv
            seg0_xa = []
            for e in range(EC):
                t_ = xavp.tile([P, SEG], BF, tag=f"xav{e}", name=f"xav{e}")
                nc.sync.dma_start(t_[:], xat_d[e * P:(e + 1) * P, 0:SEG])
                seg0_xa.append(t_)
            nc.sync.dma_start(wv_sb[0][:], wv_d[0:P, :])
            for e in range(1, EC):
                nc.sync.dma_start(wv_sb[e][:], wv_d[e * P:(e + 1) * P, :])
            # PE warm-up chain: ~50 cheap matmuls with no input deps keep the
            # HAM activity monitor busy during the initial DMA wait so the
            # first real matmuls run at 2.4GHz instead of 1.2GHz
            dumw = wvp.tile([P, 512], BF, tag="dumw", name="dumw")
            nc.vector.memset(dumw[:], 0.0)
            wps = warmp.tile([1, 512], F32, tag="wps", name="wps")
            for _ in range(48):
                nc.tensor.matmul(wps[0:1, :], ones_bf[:, 0:1], dumw[:],
                                 start=True, stop=True)
            # prefetch K-phase tables+weights (used next phase)
            cosk_sb = tabk.tile([P, cT], BF, tag="cosk", name="cosk")
            sink_sb = tabk.tile([P, cT], BF, tag="sink", name="sink")
            nc.sync.dma_start(cosk_sb[:], cosk_d[:])
            nc.sync.dma_start(sink_sb[:], sink_d[:])
            wk_sb = []
            for e in range(EC):
                t_ = wkp.tile([P, F], BF, tag=f"wk{e}", name=f"wk{e}")
                nc.sync.dma_start(t_[:], wk_d[e * P:(e + 1) * P, :])
                wk_sb.append(t_)
            for h0, hw in _cs(cT, SEG):
                if h0 == 0:
                    xa_sb = seg0_xa
                else:
                    xa_sb = []
                    for e in range(EC):
                        t_ = xavp.tile([P, SEG], BF, tag=f"xav{e}", name=f"xav{e}")
                        nc.sync.dma_start(
                            t_[:, 0:hw], xat_d[e * P:(e + 1) * P, h0:h0 + hw])
                        xa_sb.append(t_)
                for tl in range(hw // P):
                    t = (h0 // P) + tl
                    ps = psproj.tile([P, F], F32, tag="projpsv", name="projpsv")
                    for e in range(EC):
                        for ns, nw in _cs(F, 512):
                            nc.tensor.matmul(
                                ps[:, ns:ns + nw],
                                xa_sb[e][:, tl * P:(tl + 1) * P],
                                wv_sb[e][:, ns:ns + nw],
                                start=(e == 0), stop=(e == EC - 1),
                            )
                    nc.scalar.copy(v_sb[t][:], ps[:, 0:F])

        # ============ phase K: K-proj + RoPE =============================
        ktp = es_kqv.enter_context(tc.tile_pool(name="kt", bufs=1, side="right"))
        kt_sb = [ktp.tile([P, cT], BF, tag=f"kt{m}", name=f"kt{m}")
                 for m in range(HL)]
        with tc.tile_pool(name="rawk", bufs=1) as rawkp, \
                tc.tile_pool(name="tmpk", bufs=2) as tmpkp:
            first = True
            for h0, hw in _cs(cT, SEG):
                xa_sb = []
                for e in range(EC):
                    t_ = xakp.tile([P, SEG], BF, tag=f"xak{e}", name=f"xak{e}")
                    nc.sync.dma_start(
                        t_[:, 0:hw], xat_d[e * P:(e + 1) * P, h0:h0 + hw])
                    xa_sb.append(t_)
                if first:
                    # prefetch Q-phase tables+weights behind seg-0 loads
                    first = False
                    cosq_sb = tabq.tile([P, cTQ], BF, tag="cosq", name="cosq")
                    sinq_sb = tabq.tile([P, cTQ], BF, tag="sinq", name="sinq")
                    nc.sync.dma_start(cosq_sb[:], cosq_d[:])
                    nc.sync.dma_start(sinq_sb[:], sinq_d[:])
                    wq_sb = []
                    for e in range(EC):
                        t_ = wqp.tile([P, F], BF, tag=f"wq{e}", name=f"wq{e}")
                        nc.sync.dma_start(t_[:], wq_d[e * P:(e + 1) * P, :])
                        wq_sb.append(t_)
                for m in range(HL):
                    proj_rope(m, wk_sb, xa_sb, 0, h0, hw, cosk_sb,
                              sink_sb, kt_sb[m], "k", rawkp, tmpkp)
        es_xak.close()
        es_tabk.close()

        # ============ phase Q: Q-proj + RoPE (x^T in quarters) ===========
        qtp = es_kqv.enter_context(tc.tile_pool(name="qt", bufs=1, side="right"))
        qt_sb = []
        for m in range(HL):
            qt_sb.append(qtp.tile([P, cTQ], BF, tag=f"qt{m}", name=f"qt{m}"))
        with tc.tile_pool(name="rawq", bufs=1) as rawqp, \
                tc.tile_pool(name="tmpq", bufs=2) as tmpqp:
            TQH = min(512, cTQ)
            for th, (h0, hw) in enumerate(_cs(cTQ, TQH)):
                xt_sb = []
                for e in range(EC):
                    t_ = xtp.tile([P, TQH], BF, tag=f"xt{e}", name=f"xt{e}")
                    nc.sync.dma_start(
                        t_[:], xt_d[e * P:(e + 1) * P, h0:h0 + hw])
                    xt_sb.append(t_)
                for m in range(HL):
                    proj_rope(m, wq_sb, xt_sb, 0, h0, hw, cosq_sb, sinq_sb,
                              qt_sb[m], "q", rawqp, tmpqp)
        es_xt.close()
        es_tabq.close()
        es_proj.close()

        # ================= phase C: attention ============================
        FR = mybir.dt.float32r
        RPM = cTQ // P                # packed den rows per head
        es_wo = ExitStack()     # out-proj weights, loaded during attention
        wop = es_wo.enter_context(tc.tile_pool(name="wo", bufs=1))
        es_ya = ExitStack()     # first 4 gathered yt tiles (loaded in C)
        ya1p = es_ya.enter_context(tc.tile_pool(name="ya1", bufs=1))
        es_att = ExitStack()
        ptp = es_att.enter_context(tc.tile_pool(name="pt", bufs=5))
        pt2p = es_att.enter_context(tc.tile_pool(name="pt2", bufs=4))
        accp = es_att.enter_context(tc.tile_pool(name="acc", bufs=2))
        ytp = es_att.enter_context(tc.tile_pool(name="yt", bufs=2))
        dstp = es_att.enter_context(tc.tile_pool(name="dst", bufs=2))
        dnerp = es_att.enter_context(tc.tile_pool(name="dner", bufs=1))
        pss = es_att.enter_context(tc.tile_pool(name="pss", bufs=3, space="PSUM"))
        psy = es_att.enter_context(tc.tile_pool(name="psy", bufs=2, space="PSUM"))
        psd = es_att.enter_context(tc.tile_pool(name="psd", bufs=1, space="PSUM"))
        psb = es_att.enter_context(tc.tile_pool(name="psb", bufs=2, space="PSUM"))

        wo_sb = []
        for f in range(2 * F // P):
            t_ = wop.tile([P, EH], BF, tag=f"wo{f}", name=f"wo{f}")
            nc.sync.dma_start(t_[:], wo_d[f * P:(f + 1) * P, :])
            wo_sb.append(t_)

        ya_sb = []

        # The denominator reduction for q-block (m, qs) is "flushed" (its
        # ones-matmul + psum evacuations) early in the NEXT q-block, so the
        # in-order TensorE queue never waits on the VectorE add chain.
        def flush_den(pend):
            fm, fqs, fqw, facc, fyps, fyt = pend
            dps = psd.tile([1, NQ], F32, tag="dps", name="dps")
            nc.tensor.matmul(
                dps[0:1, 0:fqw],
                ones_bf[:, 0:1],
                facc[:, 0:fqw],
                start=True, stop=True,
            )
            nc.vector.tensor_copy(fyt[:, fqs:fqs + fqw], fyps[:, 0:fqw])
            dst = dstp.tile([1, NQ], F32, tag="dst", name="dst")
            nc.vector.tensor_copy(dst[0:1, 0:fqw], dps[0:1, 0:fqw])
            # scatter the denominator row into the packed layout
            # (DMA can cross partitions)
            bp = (fm % 4) * 32 + fqs // P
            c0 = (fm // 4) * P
            nc.sync.dma_start(
                den_sb[bp:bp + fqw // P, c0:c0 + P], dst[0:1, 0:fqw])

        def normalize_ship(fm, fyt):
            # head fm normalization (runs while head fm+1 attention computes)
            bp = (fm % 4) * 32
            c0 = (fm // 4) * P
            nc.vector.reciprocal(den_sb[bp:bp + RPM, c0:c0 + P],
                                 den_sb[bp:bp + RPM, c0:c0 + P])
            dner = dnerp.tile([1, cTQ], F32, tag="dner", name="dner")
            nc.sync.dma_start(dner[0:1, :],
                              den_sb[bp:bp + RPM, c0:c0 + P])
            for qs, qw in _cs(cTQ, NQ):
                dbc = psb.tile([P, NQ], F32, tag="dbc", name="dbc")
                nc.tensor.matmul(
                    dbc[:, 0:qw],
                    ones_fr[0:1, :].bitcast(FR),
                    dner[0:1, qs:qs + qw].bitcast(FR),
                    start=True, stop=True,
                )
                nc.vector.tensor_mul(
                    fyt[:, qs:qs + qw],
                    fyt[:, qs:qs + qw],
                    dbc[:, 0:qw],
                )
            # ship normalized head to the pair-exchange buffer; fire the
            # block's AllGather once both of its heads have landed
            blk, ml = divmod(fm, HH)
            nc.sync.dma_start(agin[blk][ml * P:(ml + 1) * P, :], fyt[:])
            if ml == HH - 1:
                nc.gpsimd.collective_compute(
                    "AllGather",
                    mybir.AluOpType.bypass,
                    replica_groups=groups,
                    ins=[agin[blk][:]],
                    outs=[agout[blk][:]],
                )
            if fm == 3:
                # first gathered block (4 tiles): DMA in during heads 4-7
                for f in range(2 * HH):
                    t_ = ya1p.tile([P, cTQ], BF, tag=f"ya{f}", name=f"ya{f}")
                    nc.sync.dma_start(t_[:], agout[0][f * P:(f + 1) * P, :])
                    ya_sb.append(t_)

        pending = None            # last q-block awaiting its denominator flush
        pending_head = None       # last head awaiting normalization
        for m in range(HL):
            yt_t = ytp.tile([P, cTQ], BF, tag="yt", name=f"yt{m}")
            for qs, qw in _cs(cTQ, NQ):
                yps = psy.tile([P, NQ], F32, tag="yps", name="yps")
                acc = None
                prev = None
                for kc in range(TC):
                    sps = pss.tile([P, NQ], F32, tag="sps", name="sps")
                    nc.tensor.matmul(
                        sps[:, 0:qw],
                        kt_sb[m][:, kc * P:(kc + 1) * P],
                        qt_sb[m][:, qs:qs + qw],
                        start=True, stop=True,
                    )
                    pt = ptp.tile([P, NQ], BF, tag="pt", name="pt")
                    nc.scalar.activation(
                        pt[:, 0:qw], sps[:, 0:qw],
                        mybir.ActivationFunctionType.Exp,
                        bias=mb_sb[:, kc:kc + 1], scale=SCALE,
                    )
                    nc.tensor.matmul(
                        yps[:, 0:qw],
                        v_sb[kc][:, m * P:(m + 1) * P],
                        pt[:, 0:qw],
                        start=(kc == 0), stop=(kc == TC - 1),
                    )
                    if kc == 1:
                        # previous block's denominator + evacuations go on
                        # the queues here, after this block's first matmuls
                        if pending is not None:
                            flush_den(pending)
                            pending = None
                        if pending_head is not None:
                            normalize_ship(*pending_head)
                            pending_head = None
                    if kc % 2 == 1:
                        # denominator: pairwise add + running accumulator,
                        # interleaved with the kc loop; all on VectorE (the
                        # GpSimd queue is blocked ~20us per collective
                        # trigger, which would back-pressure pools)
                        pt2 = pt2p.tile([P, NQ], BF, tag="pt2", name="pt2")
                        nc.vector.tensor_add(pt2[:, 0:qw], prev[:, 0:qw],
                                             pt[:, 0:qw])
                        if acc is None:
                            acc = pt2
                        else:
                            nacc = accp.tile([P, NQ], BF, tag="acc", name="acc")
                            nc.vector.tensor_add(nacc[:, 0:qw], acc[:, 0:qw],
                                                 pt2[:, 0:qw])
                            acc = nacc
                    prev = pt
                pending = (m, qs, qw, acc, yps, yt_t)
            pending_head = (m, yt_t)
        flush_den(pending)
        normalize_ship(*pending_head)
        es_att.close()
        es_kqv.close()

        # ================= phase D: out-projection =======================
        # out^T[EH, q] = Wo'^T @ ya. The f contraction for each ms block is
        # emitted in three sweeps over all 8 PSUM banks — f 0..3 (in SBUF
        # since mid-attention), f 4..11 (gathered blocks 1-2, DMA'd at D
        # start), f 12..15 (gathered block 3) — so ~20us of matmuls on
        # already-arrived data hide the final AllGather and its loads.
        NT = EH // P
        NF = 2 * HL
        with tc.tile_pool(name="ya2", bufs=1) as ya2p, \
                tc.tile_pool(name="oev", bufs=4) as oevp, \
                tc.tile_pool(name="pso", bufs=8, space="PSUM") as pso:
            for f in range(2 * HH, NF):
                blk, r = divmod(f, 2 * HH)
                t_ = ya2p.tile([P, cTQ], BF, tag=f"yb{f}", name=f"yb{f}")
                nc.sync.dma_start(t_[:], agout[blk][r * P:(r + 1) * P, :])
                ya_sb.append(t_)
            sweeps = [(0, 4), (4, 12), (12, NF)]
            for ms, mw in _cs(cTQ, 512):
                opss = []
                for si, (f0, f1) in enumerate(sweeps):
                    for n in range(NT):
                        if si == 0:
                            ops = pso.tile([P, 512], F32, tag="ops", name="ops")
                            opss.append(ops)
                        else:
                            ops = opss[n]
                        for f in range(f0, f1):
                            nc.tensor.matmul(
                                ops[:, 0:mw],
                                wo_sb[f][:, n * P:(n + 1) * P],
                                ya_sb[f][:, ms:ms + mw],
                                start=(f == 0), stop=(f == NF - 1),
                            )
                        if si == len(sweeps) - 1:
                            oev = oevp.tile([P, 512], BF, tag="oev", name="oev")
                            nc.scalar.copy(oev[:, 0:mw], ops[:, 0:mw])
                            nc.sync.dma_start(
                                out_d[n * P:(n + 1) * P, ms:ms + mw],
                                oev[:, 0:mw])
        es_ya.close()
        es_wo.close()

    return nc


# ---------------------------------------------------------------------------
# host side
# ---------------------------------------------------------------------------

def _rope_tables():
    inv_freq = 1.0 / (THETA ** (np.arange(0, D, 2, dtype=np.float32) / D))
    t = np.arange(BLOCK, dtype=np.float32)
    freqs = np.einsum("i,j->ij", t, inv_freq).astype(np.float32)
    emb = np.concatenate([freqs, freqs], axis=-1)
    return np.cos(emb).astype(np.float32), np.sin(emb).astype(np.float32)


_NC_CACHE = {}


def _get_compiled():
    if "nc" not in _NC_CACHE:
        nc = build_nc()
        nc.compile()
        _NC_CACHE["nc"] = nc
    return _NC_CACHE["nc"]


def _bf(a):
    return np.ascontiguousarray(a).astype(BF16NP)


def prepare_in_maps(x, xall, posx, posxall, mask, Wq, Wk, Wv, Wo):
    x = np.asarray(x, dtype=np.float32)
    xall = np.asarray(xall, dtype=np.float32)
    posx = np.asarray(posx)
    posxall = np.asarray(posxall)
    mask = np.asarray(mask)
    Wq = np.asarray(Wq, dtype=np.float32)
    Wk = np.asarray(Wk, dtype=np.float32)
    Wv = np.asarray(Wv, dtype=np.float32)
    Wo = np.asarray(Wo, dtype=np.float32)

    cos_t, sin_t = _rope_tables()
    sign = np.ones((1, D), np.float32)
    sign[0, : D // 2] = -1.0

    F = (H * D) // 2  # 1024: per-core head-shard width
    FB = 2 * D        # 256: AllGather block (2 heads)
    # AllGather block order: [A blk_i, B blk_i] for i in 0..3, where A/B are
    # the pair's rank-0/rank-1 feature halves of Wo's rows
    Wo_perm = np.concatenate(
        [w for i in range(4)
         for w in (Wo[i * FB:(i + 1) * FB], Wo[F + i * FB:F + (i + 1) * FB])],
        axis=0)

    in_maps = []
    for c in range(N_CORES):
        b, hg = c // 2, c % 2
        sl = slice(hg * F, (hg + 1) * F)
        cosq = _bf(cos_t[posx[b]].T)                    # [128, TQ]
        sinq = _bf((sin_t[posx[b]] * sign).T)
        cosk = _bf(cos_t[posxall[b]].T)
        sink = _bf((sin_t[posxall[b]] * sign).T)
        mb = np.where(mask[b], np.float32(-60.0), np.float32(0.0))
        mb = np.ascontiguousarray(mb.reshape(T // P, P).T)  # [128, TC]
        in_maps.append({
            "xt": _bf(x[b].T),
            "xat": _bf(xall[b].T),
            "wq": _bf(Wq[:, sl]),
            "wk": _bf(Wk[:, sl]),
            "wv": _bf(Wv[:, sl]),
            "wo": _bf(Wo_perm[:, hg * (E // 2):(hg + 1) * (E // 2)]),
            "cosq": cosq, "sinq": sinq, "cosk": cosk, "sink": sink,
            "mbias": mb.astype(np.float32),
        })
    return in_maps


def assemble_out(results):
    # core (b, hg) computed out^T for E columns [hg*E/2, (hg+1)*E/2)
    EH = E // 2
    out = np.empty((B, TQ, E), np.float32)
    for b in range(B):
        for hg in range(2):
            half = results[2 * b + hg]["out"].astype(np.float32)
            out[b][:, hg * EH:(hg + 1) * EH] = half.T
    return out


def kernel(x, xall, posx, posxall, mask, Wq, Wk, Wv, Wo):
    from concourse.bass_utils import run_bass_kernel_spmd

    in_maps = prepare_in_maps(x, xall, posx, posxall, mask, Wq, Wk, Wv, Wo)
    nc = _get_compiled()
    res = run_bass_kernel_spmd(nc, in_maps, list(range(N_CORES)), trace=False)
    return assemble_out(res.results)
